# revision 29
# baseline (speedup 1.0000x reference)
"""GAT (graph attention) kernel for Trainium2, 8-core SPMD.

Strategy:
  - Nodes sharded 8 ways (12500/core, padded to 12544 = 98*128).
  - Stage A: per 128-node tile, one PE matmul against [W | B] (136 cols)
    produces h (128) + e_src (4) + e_dst (4) in one PSUM tile; B = the
    per-head attention vectors folded through W (built on device once).
    Table row (512B bf16): [h bf16 x128 | e_dst f32 x4 (bitcast) | 0 pad].
  - Stage B: AllGather slabs -> full 100352-row table per core.
  - Stage C: nodes are Hilbert-ordered by window-count profile so tiles are
    homogeneous; tiles are packed into variable-size groups under a gather-
    buffer cap; per (group, window) one dma_gather call (window-major
    contiguous output block, int16 idxs). The table is f32-typed (h stored
    as bf16 pairs bitcast into f32 slots, e_dst as true f32): bf16-typed
    AllGathers canonicalize 16-bit lanes that alias bf16 NaN patterns.
    Scores assembled per window segment, softmax per tile, exp expanded on
    the ACT engine (bf16), weighted h summed by a pairwise add tree on DVE
    (2x bf16 mode), ELU, out.
  - The per-group slot table J is data-dependent: the bass program is built
    and compiled on first kernel() call (cached by J-table hash).

kernel(**inputs) does host-side sharding/index prep only (no FP math on
tensor data), runs the SPMD program, and reassembles the full output.
"""
import sys

if "/opt/trn_rl_repo" not in sys.path:
    sys.path.insert(0, "/opt/trn_rl_repo")

import hashlib
import numpy as np

N, DEG, K, F_IN, F_OUT = 100000, 16, 4, 128, 32
KF = K * F_OUT            # 128
N_CORES = 8
S = N // N_CORES          # 12500
P = 128
NT = (S + P - 1) // P     # 98
SP = NT * P               # 12544
NTAB = N_CORES * SP       # 100352
EL = 256                  # bf16 elements per table row (512B)
ELF = 128                 # f32 elements per table row (512B)
HF = KF // 2              # 64: f32 slots holding the h bf16 pairs
RHS = KF + 2 * K          # 136: [h | e_src k0..3 | e_dst k0..3]
BOUNDS = (0, 2 * SP, 4 * SP, 6 * SP, NTAB)   # shard-pair aligned (25088)
NW = 4
GCAP = 64                 # max gather-buffer columns per group (x512B)
# dummy rows: first pad row of shards 0,2,4,6 (one inside each window)
DUMMY = (S, 2 * SP + S, 4 * SP + S, 6 * SP + S)
NEG_SLOPE = 0.01
NEG_BIG = -1.0e30


def build_nc(J, gsizes, n_cores=N_CORES, nt=NT, distributed=True,
             dump=False):
    """J: [ng, NW] per-group per-window slot counts (uniform across cores);
    gsizes: tiles per group. Builds and compiles the SPMD program."""
    from contextlib import ExitStack

    import concourse.bass as bass
    import concourse.tile as tile
    from concourse import bacc, mybir
    from concourse.masks import make_identity

    f32 = mybir.dt.float32
    bf16 = mybir.dt.bfloat16
    i16 = mybir.dt.int16
    AF = mybir.ActivationFunctionType
    sp = nt * P
    J = np.asarray(J, np.int64)
    Jt = J.sum(axis=1)            # slots per tile, per group
    NG = len(gsizes)
    CTOT = int(sum(gsizes[g] * J[g].sum() for g in range(NG))) * 8

    nc = bacc.Bacc("TRN2", target_bir_lowering=False, debug=False,
                   num_devices=n_cores, num_swdge_queues=4)

    xs = nc.dram_tensor("xs", [sp, F_IN], f32, kind="ExternalInput")
    wt = nc.dram_tensor("wt", [F_IN, KF], f32, kind="ExternalInput")
    avec = nc.dram_tensor("avec", [2, KF], f32, kind="ExternalInput")
    idxin = nc.dram_tensor("idxin", [P, CTOT], i16, kind="ExternalInput")
    padfill = nc.dram_tensor("padfill", [sp - S if sp > S else 1, 4], f32,
                             kind="ExternalInput")
    out = nc.dram_tensor("out", [sp, KF], f32, kind="ExternalOutput")

    m_out = z_out = s_out = hfe_out = g0_out = es_out = hse_out = None
    JTMAX = int(Jt.max())
    if dump:
        m_out = nc.dram_tensor("m_out", [sp, K], f32, kind="ExternalOutput")
        z_out = nc.dram_tensor("z_out", [sp, K], f32, kind="ExternalOutput")
        s_out = nc.dram_tensor("s_out", [sp, K * JTMAX], f32,
                               kind="ExternalOutput")
        hfe_out = nc.dram_tensor("hfe_out", [NTAB, 4], f32,
                                 kind="ExternalOutput")
        tc0 = int(gsizes[0]) * int(Jt[0])
        g0_out = nc.dram_tensor("g0_out", [P, tc0 * 4], f32,
                                kind="ExternalOutput")
        es_out = nc.dram_tensor("es_out", [sp, K], f32,
                                kind="ExternalOutput")
        hse_out = nc.dram_tensor("hse_out", [sp, 4], f32,
                                 kind="ExternalOutput")
    he_shard = nc.dram_tensor("he_shard", [sp, ELF], f32, kind="Internal")
    he_full = nc.dram_tensor("he_full", [NTAB, ELF], f32, kind="Internal",
                             addr_space="Shared" if distributed else "Local")

    with tile.TileContext(nc) as tc, ExitStack() as ctx:
        consts = ctx.enter_context(tc.tile_pool(name="consts", bufs=1))
        sa = ctx.enter_context(tc.tile_pool(name="sa", bufs=3))
        sa_ps = ctx.enter_context(tc.tile_pool(name="sa_ps", bufs=2, space="PSUM"))
        sc = ctx.enter_context(tc.tile_pool(name="sc", bufs=2))
        scg = ctx.enter_context(tc.tile_pool(name="scg", bufs=3))
        sci = ctx.enter_context(tc.tile_pool(name="sci", bufs=3))

        ident = consts.tile([P, P], f32)
        make_identity(nc, ident[:])
        # rhs_cat = [wt | B]; B[i, s*K+k] = sum_f wt[i, k*32+f] * a_s[k, f]
        rhs_cat = consts.tile([P, RHS], f32)
        nc.sync.dma_start(rhs_cat[:, 0:KF], wt.ap())
        av_sb = consts.tile([P, 2 * KF], f32)
        nc.sync.dma_start(av_sb[:], bass.AP(avec, 0, [[0, P], [1, 2 * KF]]))
        tmp_b = consts.tile([P, 2 * KF], f32)
        for s in range(2):
            nc.vector.tensor_mul(
                tmp_b[:, s * KF:(s + 1) * KF], rhs_cat[:, 0:KF],
                av_sb[:, s * KF:(s + 1) * KF])
        nc.vector.reduce_sum(
            rhs_cat[:, KF:RHS],
            tmp_b[:].rearrange("p (s k f) -> p s k f", s=2, f=F_OUT),
            axis=mybir.AxisListType.X)
        es_sb = consts.tile([P, nt * K], f32)

        # ---- Stage A (two tiles per iteration; batched copies/IO) ----
        assert nt % 2 == 0
        for t2 in range(0, nt, 2):
            x_t2 = sa.tile([P, 2 * F_IN], f32, tag="x")
            xv = x_t2[:].rearrange("p (t f) -> p t f", f=F_IN)
            nc.sync.dma_start(
                xv, xs.ap()[t2 * P:(t2 + 2) * P, :]
                    .rearrange("(t p) f -> p t f", p=P))
            xt_ps2 = sa_ps.tile([P, 2 * P], f32, tag="xt")
            for ti in range(2):
                nc.tensor.transpose(out=xt_ps2[:, ti * P:(ti + 1) * P],
                                    in_=xv[:, ti, :], identity=ident[:])
            xt_sb2 = sa.tile([P, 2 * P], f32, tag="xt_sb")
            nc.scalar.copy(xt_sb2[:], xt_ps2[:])
            he_ps2 = sa_ps.tile([P, 2 * RHS], f32, tag="he")
            for ti in range(2):
                nc.tensor.matmul(he_ps2[:, ti * RHS:(ti + 1) * RHS],
                                 lhsT=xt_sb2[:, ti * P:(ti + 1) * P],
                                 rhs=rhs_cat[:], start=True, stop=True)
            he_t2 = sa.tile([P, 2 * ELF], f32, tag="het")
            hev = he_t2[:].rearrange("p (t e) -> p t e", e=ELF)
            hpv = he_ps2[:].rearrange("p (t r) -> p t r", r=RHS)
            nc.gpsimd.memset(he_t2[:], 0.0)
            nc.scalar.copy(hev[:, :, 0:HF].bitcast(bf16), hpv[:, :, 0:KF])
            nc.vector.tensor_copy(
                hev[:, :, HF:HF + K], hpv[:, :, KF + K:RHS])
            nc.vector.tensor_copy(
                es_sb[:, t2 * K:(t2 + 2) * K]
                    .rearrange("p (t k) -> p t k", k=K),
                hpv[:, :, KF:KF + K])
            nc.sync.dma_start(
                he_shard.ap()[t2 * P:(t2 + 2) * P, :]
                    .rearrange("(t p) e -> p t e", p=P), hev)
        # pad rows are window dummies: e_dst <- -1e30 (after slab writes)
        npad = sp - S
        if npad > 0:
            pf = consts.tile([npad, 4], f32)
            nc.sync.dma_start(pf[:], padfill.ap())
            nc.sync.dma_start(he_shard.ap()[S:sp, HF:HF + K], pf[:])

        # ---- Stage B ----
        if distributed:
            nc.gpsimd.collective_compute(
                "AllGather", mybir.AluOpType.bypass,
                replica_groups=[list(range(n_cores))],
                ins=[he_shard.ap()], outs=[he_full.ap()])
        else:
            for t in range(nt):
                cp = sa.tile([P, ELF], f32, tag="cp")
                nc.sync.dma_start(cp[:], he_shard.ap()[t * P:(t + 1) * P, :])
                nc.sync.dma_start(he_full.ap()[t * P:(t + 1) * P, :], cp[:])

        if dump:
            nc.sync.dma_start(
                hse_out.ap(),
                bass.AP(he_shard, HF, [[ELF, sp], [1, 4]]))
            HB = NTAB // 4
            for q in range(4):
                nc.sync.dma_start(
                    hfe_out.ap()[q * HB:(q + 1) * HB, :],
                    bass.AP(he_full, q * HB * ELF + HF, [[ELF, HB], [1, 4]]))
            nc.sync.dma_start(
                es_out.ap().rearrange("(t p) k -> p t k", p=P),
                es_sb[:].rearrange("p (t k) -> p t k", k=K))
        # ---- Stage C ----
        coff = 0
        ncall = 0
        tbase = 0
        for g in range(NG):
            gs = gsizes[g]
            jt = int(Jt[g])
            totcol = gs * jt
            idx_g = sci.tile([P, totcol * 8], i16, tag="idx")
            nc.sync.dma_start(idx_g[:], idxin.ap()[:, coff:coff + totcol * 8])
            coff += totcol * 8
            gbuf = scg.tile([P, totcol * ELF], f32, tag="g")
            g3 = gbuf[:].rearrange("p (c e) -> p c e", e=ELF)
            # window-major gather blocks: block w at column gs*sum(J[g,:w])
            bw = 0
            ioff = 0
            for w in range(NW):
                jw = int(J[g, w])
                if jw == 0:
                    continue
                nidx = gs * jw * P
                nc.gpsimd.dma_gather(
                    out_ap=g3[:, bw:bw + gs * jw, :],
                    in_ap=he_full.ap()[BOUNDS[w]:BOUNDS[w + 1], :],
                    idxs_ap=idx_g[:, ioff:ioff + gs * jw * 8],
                    num_idxs=nidx, num_idxs_reg=nidx, elem_size=ELF,
                    single_packet=False, queue_num=ncall % 4)
                ncall += 1
                bw += gs * jw
                ioff += gs * jw * 8
            if dump and g == 0:
                nc.sync.dma_start(g0_out.ap(), g3[:, :, HF:HF + K])
            # group-wide score assembly: sag[p, t, k, d]
            s_allg = sc.tile([P, gs * K * jt], f32, tag="s0")
            sag = s_allg[:].rearrange("p (t k d) -> p t k d", k=K, d=jt)
            esg = es_sb[:, tbase * K:(tbase + gs) * K].rearrange(
                "p (t k) -> p t k", k=K).unsqueeze(-1)
            bw = 0
            c0 = 0
            for w in range(NW):
                jw = int(J[g, w])
                if jw == 0:
                    continue
                eb = g3[:, bw:bw + gs * jw, HF:HF + K].rearrange(
                    "p (t d) k -> p t k d", d=jw)
                nc.vector.tensor_add(
                    sag[:, :, :, c0:c0 + jw], eb,
                    esg.to_broadcast([P, gs, K, jw]))
                bw += gs * jw
                c0 += jw
            s1g = sc.tile([P, gs * K * jt], f32, tag="s1")
            nc.vector.scalar_tensor_tensor(
                s1g[:], s_allg[:], NEG_SLOPE, s_allg[:],
                op0=mybir.AluOpType.mult, op1=mybir.AluOpType.max)
            s1v = s1g[:].rearrange("p (tk d) -> p tk d", d=jt)
            mg = sc.tile([P, gs * K], f32, tag="m")
            nc.vector.reduce_max(mg[:], s1v, axis=mybir.AxisListType.X)
            s2g = sc.tile([P, gs * K * jt], f32, tag="s2")
            nc.vector.tensor_sub(
                s2g[:].rearrange("p (tk d) -> p tk d", d=jt), s1v,
                mg[:].unsqueeze(-1).to_broadcast([P, gs * K, jt]))
            zg = sc.tile([P, gs * K], f32, tag="z")
            prs = sc.tile([P, gs * K * jt], f32, tag="prs")
            nc.scalar.activation(prs[:], s2g[:], AF.Exp)
            nc.vector.reduce_sum(
                zg[:], prs[:].rearrange("p (tk d) -> p tk d", d=jt),
                axis=mybir.AxisListType.X)
            vall = sc.tile([P, gs * KF], bf16, tag="vall")
            for ti in range(gs):
                t = tbase + ti
                # expanded exp on ACT: prx[p, d, (k f)] = exp(s2[p, k, d])
                prx = sc.tile([P, jt * KF], bf16, tag="prx")
                nc.scalar.activation(
                    prx[:].rearrange("p (d k f) -> p d k f", k=K, f=F_OUT),
                    s2g[:, ti * K * jt:(ti + 1) * K * jt]
                        .rearrange("p (k d) -> p d k", d=jt)
                        .unsqueeze(-1).to_broadcast([P, jt, K, F_OUT]),
                    AF.Exp)
                if dump:
                    nc.sync.dma_start(
                        s_out.ap()[t * P:(t + 1) * P, 0:K * jt],
                        s_allg[:, ti * K * jt:(ti + 1) * K * jt])
                # weighted h: wg[p, d, kf] = g_h * prx
                wg = sc.tile([P, jt * KF], bf16, tag="wg")
                wgv = wg[:].rearrange("p (d e) -> p d e", e=KF)
                prxv = prx[:].rearrange("p (d e) -> p d e", e=KF)
                bw = 0
                c0 = 0
                for w in range(NW):
                    jw = int(J[g, w])
                    if jw == 0:
                        continue
                    nc.vector.tensor_mul(
                        wgv[:, c0:c0 + jw, :],
                        g3[:, bw + ti * jw: bw + (ti + 1) * jw,
                           0:HF].bitcast(bf16),
                        prxv[:, c0:c0 + jw, :])
                    bw += gs * jw
                    c0 += jw
                # pairwise add tree over slots (bf16 2x mode); final level
                # lands in this tile's vall slice
                buf, width, lvl = wgv, jt, 0
                while width > 1:
                    h2, r = divmod(width, 2)
                    if h2 + r == 1:
                        nxt = vall[:, ti * KF:(ti + 1) * KF].rearrange(
                            "p (d e) -> p d e", e=KF)
                    else:
                        nxt_t = sc.tile([P, (h2 + r) * KF], bf16,
                                        tag=f"tr{lvl}")
                        nxt = nxt_t[:].rearrange("p (d e) -> p d e", e=KF)
                    nc.vector.tensor_add(
                        nxt[:, 0:h2, :],
                        buf[:, 0:2 * h2:2, :], buf[:, 1:2 * h2:2, :])
                    if r:
                        nc.vector.tensor_copy(
                            nxt[:, h2, :], buf[:, 2 * h2, :])
                    buf, width = nxt, h2 + r
                    lvl += 1
            # group-wide normalize + ELU + store
            rzg = sc.tile([P, gs * K], f32, tag="rz")
            nc.vector.reciprocal(rzg[:], zg[:])
            if dump:
                nc.sync.dma_start(
                    m_out.ap()[tbase * P:(tbase + gs) * P, :]
                        .rearrange("(t p) k -> p t k", p=P),
                    mg[:].rearrange("p (t k) -> p t k", k=K))
                nc.sync.dma_start(
                    z_out.ap()[tbase * P:(tbase + gs) * P, :]
                        .rearrange("(t p) k -> p t k", p=P),
                    zg[:].rearrange("p (t k) -> p t k", k=K))
            og = sc.tile([P, gs * KF], f32, tag="o")
            nc.vector.tensor_mul(
                og[:].rearrange("p (tk f) -> p tk f", f=F_OUT),
                vall[:].rearrange("p (tk f) -> p tk f", f=F_OUT),
                rzg[:].unsqueeze(-1).to_broadcast([P, gs * K, F_OUT]))
            t1 = sc.tile([P, gs * KF], f32, tag="t1")
            nc.vector.tensor_scalar_min(t1[:], og[:], 0.0)
            e1 = sc.tile([P, gs * KF], f32, tag="e1")
            nc.scalar.activation(e1[:], t1[:], AF.Exp)
            r_ = sc.tile([P, gs * KF], f32, tag="r")
            nc.vector.tensor_scalar_max(r_[:], og[:], 0.0)
            ot = sc.tile([P, gs * KF], f32, tag="ot")
            nc.vector.scalar_tensor_tensor(
                ot[:], e1[:], -1.0, r_[:],
                op0=mybir.AluOpType.add, op1=mybir.AluOpType.add)
            nc.sync.dma_start(
                out.ap()[tbase * P:(tbase + gs) * P, :]
                    .rearrange("(t p) f -> p t f", p=P),
                ot[:].rearrange("p (t f) -> p t f", f=KF))
            tbase += gs

    nc.compile()
    return nc


def _hilbert_d(coords, bits=5):
    """Hilbert distance of [n, d] integer coords (Skilling transpose)."""
    n, nd = coords.shape
    X = coords.T.astype(np.uint32).copy()
    M = np.uint32(1 << (bits - 1))
    Q = M
    while Q > 1:
        Pq = Q - 1
        for i in range(nd):
            t = (X[i] & Q) > 0
            X[0] = np.where(t, X[0] ^ Pq, X[0])
            m = np.where(t, np.uint32(0), Pq)
            tt = (X[0] ^ X[i]) & m
            X[0] ^= tt
            X[i] ^= tt
        Q >>= 1
    for i in range(1, nd):
        X[i] ^= X[i - 1]
    t2 = np.zeros(n, np.uint32)
    Q = M
    while Q > 1:
        t2 = np.where((X[nd - 1] & Q) > 0, t2 ^ (Q - 1), t2)
        Q >>= 1
    for i in range(nd):
        X[i] ^= t2
    out = np.zeros(n, np.int64)
    for b in range(bits):
        for i in range(nd):
            out |= np.int64(((X[i] >> (bits - 1 - b)) & 1).astype(np.int64)
                            ) << ((bits - 1 - b) * nd + (nd - 1 - i))
    return out


def host_plan(nbr):
    """Hilbert node ordering per core, per-group window slot table J,
    per-core idx buffers (window-major grouped gather layout)."""
    nbr = np.asarray(nbr).astype(np.int64)
    src_core = nbr // S
    win = src_core // 2                                     # [N, DEG] in 0..3
    orders = []
    cnts = []
    for c in range(N_CORES):
        w = win[c * S:(c + 1) * S]
        cnt = np.stack([(w == q).sum(1) for q in range(NW)], 1)  # [S, NW]
        order = np.argsort(_hilbert_d(cnt), kind="stable")
        orders.append(order)
        cnts.append(cnt)
    # table row of neighbor j = c_j*SP + inv_order_{c_j}(j % S)
    inv = np.empty(N, np.int64)
    for c in range(N_CORES):
        inv[c * S + orders[c]] = np.arange(S)
    rows = (src_core * SP + inv[nbr]).astype(np.int64)
    percore = []
    M = np.zeros((NT, NW), np.int64)    # per-tile cross-core window maxima
    for c in range(N_CORES):
        r = np.sort(rows[c * S:(c + 1) * S], axis=1)[orders[c]]  # [S, 16]
        cnt = cnts[c][orders[c]]
        rs = np.concatenate([r, np.zeros((SP - S, DEG), np.int64)])
        cs = np.concatenate([cnt, np.zeros((SP - S, NW), np.int64)])
        start = np.concatenate(
            [np.zeros((SP, 1), np.int64), np.cumsum(cs, 1)[:, :-1]], 1)
        percore.append((rs, cs, start))
        M = np.maximum(M, cs.reshape(NT, P, NW).max(1))
    # greedy variable grouping under the gather-buffer column cap
    groups = []
    t0 = 0
    while t0 < NT:
        gs = 1
        Jg = M[t0].copy()
        while t0 + gs < NT:
            Jn = np.maximum(Jg, M[t0 + gs])
            if (gs + 1) * Jn.sum() > GCAP:
                break
            Jg = Jn
            gs += 1
        groups.append((t0, gs, Jg))
        t0 += gs
    gsizes = [gs for _, gs, _ in groups]
    J = np.stack([Jg for _, _, Jg in groups])               # [ng, NW]
    NG = len(groups)
    idxbufs = []
    for c in range(N_CORES):
        rs, cs, start = percore[c]
        segs = []
        for g in range(NG):
            t0g, gs, _ = groups[g]
            lo = t0g * P
            for w in range(NW):
                jw = int(J[g, w])
                if jw == 0:
                    continue
                rt = rs[lo:lo + gs * P]                    # [gs*128, 16]
                ct = cs[lo:lo + gs * P, w:w + 1]
                st = start[lo:lo + gs * P, w:w + 1]
                jj = np.arange(jw)[None, :]
                take = st + jj
                valid = jj < ct
                vals = np.where(
                    valid,
                    np.take_along_axis(
                        rt, np.minimum(take, DEG - 1).astype(np.int64), 1),
                    DUMMY[w]).astype(np.int64) - BOUNDS[w]
                # [gs*128, jw] -> stream: for t: for j: for p
                v3 = vals.reshape(gs, P, jw).transpose(0, 2, 1)
                lin = v3.reshape(-1)                        # [gs*jw*128]
                seg = lin.reshape(-1, 16).T.astype(np.int16)  # [16, gs*jw*8]
                segs.append(seg)
        buf16 = np.concatenate(segs, axis=1)
        idxbufs.append(np.ascontiguousarray(np.tile(buf16, (8, 1))))
    return J, gsizes, orders, idxbufs


def prep_inputs(X, W, a, nbr):
    X = np.asarray(X, dtype=np.float32)
    W = np.asarray(W, dtype=np.float32)
    a = np.asarray(a, dtype=np.float32)
    J, gsizes, orders, idxbufs = host_plan(nbr)
    wt = np.ascontiguousarray(W.transpose(2, 0, 1).reshape(F_IN, KF))
    avec = np.ascontiguousarray(
        np.stack([a[:, 0, :F_OUT].reshape(KF), a[:, 0, F_OUT:].reshape(KF)]))
    pf = np.ascontiguousarray(
        np.full((max(SP - S, 1), 4), NEG_BIG, dtype=np.float32))
    in_maps = []
    for c in range(N_CORES):
        xs = np.zeros((SP, F_IN), dtype=np.float32)
        xs[:S] = X[c * S:(c + 1) * S][orders[c]]
        in_maps.append({"xs": xs, "wt": wt, "avec": avec, "idxin": idxbufs[c],
                        "padfill": pf})
    return J, gsizes, orders, in_maps


_NC_CACHE = {}


def kernel(X, W, a, nbr):
    from concourse.bass_utils import run_bass_kernel_spmd

    J, gsizes, orders, in_maps = prep_inputs(X, W, a, nbr)
    key = hashlib.sha1(
        J.tobytes() + np.asarray(gsizes, np.int64).tobytes()).hexdigest()
    if key not in _NC_CACHE:
        _NC_CACHE[key] = build_nc(J, gsizes)
    nc = _NC_CACHE[key]
    res = run_bass_kernel_spmd(nc, in_maps, core_ids=list(range(N_CORES)))
    out = np.empty((N, KF), dtype=np.float32)
    for c in range(N_CORES):
        out[c * S + orders[c]] = res.results[c]["out"][:S]
    return out


# revision 30
# speedup vs baseline: 1.2487x; 1.2487x over previous
"""GAT (graph attention) kernel for Trainium2, 8-core SPMD.

Strategy:
  - Nodes sharded 8 ways (12500/core, padded to 12544 = 98*128).
  - Stage A: per 128-node tile, one PE matmul against [W | B] (136 cols)
    produces h (128) + e_src (4) + e_dst (4) in one PSUM tile; B = the
    per-head attention vectors folded through W (built on device once).
    Table row (512B bf16): [h bf16 x128 | e_dst f32 x4 (bitcast) | 0 pad].
  - Stage B: AllGather slabs -> full 100352-row table per core.
  - Stage C: nodes are Hilbert-ordered by window-count profile so tiles are
    homogeneous; tiles are packed into variable-size groups under a gather-
    buffer cap; per (group, window) one dma_gather call (window-major
    contiguous output block, int16 idxs). The table is f32-typed (h stored
    as bf16 pairs bitcast into f32 slots, e_dst as true f32): bf16-typed
    AllGathers canonicalize 16-bit lanes that alias bf16 NaN patterns.
    Scores assembled per window segment, softmax per tile, exp expanded on
    the ACT engine (bf16), weighted h summed by a pairwise add tree on DVE
    (2x bf16 mode), ELU, out.
  - The per-group slot table J is data-dependent: the bass program is built
    and compiled on first kernel() call (cached by J-table hash).

kernel(**inputs) does host-side sharding/index prep only (no FP math on
tensor data), runs the SPMD program, and reassembles the full output.
"""
import sys

if "/opt/trn_rl_repo" not in sys.path:
    sys.path.insert(0, "/opt/trn_rl_repo")

import hashlib
import numpy as np

N, DEG, K, F_IN, F_OUT = 100000, 16, 4, 128, 32
KF = K * F_OUT            # 128
N_CORES = 8
S = N // N_CORES          # 12500
P = 128
NT = (S + P - 1) // P     # 98
SP = NT * P               # 12544
NTAB = N_CORES * SP       # 100352
EL = 256                  # bf16 elements per table row (512B)
ELF = 128                 # f32 elements per table row (512B)
HF = KF // 2              # 64: f32 slots holding the h bf16 pairs
RHS = KF + 2 * K          # 136: [h | e_src k0..3 | e_dst k0..3]
BOUNDS = (0, 2 * SP, 4 * SP, 6 * SP, NTAB)   # shard-pair aligned (25088)
NW = 4
GCAP = 64                 # max gather-buffer columns per group (x512B)
# dummy rows: first pad row of shards 0,2,4,6 (one inside each window)
DUMMY = (S, 2 * SP + S, 4 * SP + S, 6 * SP + S)
NEG_SLOPE = 0.01
NEG_BIG = -1.0e30


def build_nc(J, gsizes, n_cores=N_CORES, nt=NT, distributed=True,
             dump=False):
    """J: [ng, NW] per-group per-window slot counts (uniform across cores);
    gsizes: tiles per group. Builds and compiles the SPMD program."""
    from contextlib import ExitStack

    import concourse.bass as bass
    import concourse.tile as tile
    from concourse import bacc, mybir
    from concourse.masks import make_identity

    f32 = mybir.dt.float32
    bf16 = mybir.dt.bfloat16
    i16 = mybir.dt.int16
    AF = mybir.ActivationFunctionType
    sp = nt * P
    J = np.asarray(J, np.int64)
    Jt = J.sum(axis=1)            # slots per tile, per group
    NG = len(gsizes)
    CTOT = int(sum(gsizes[g] * J[g].sum() for g in range(NG))) * 8

    nc = bacc.Bacc("TRN2", target_bir_lowering=False, debug=False,
                   num_devices=n_cores, num_swdge_queues=4)

    xs = nc.dram_tensor("xs", [sp, F_IN], f32, kind="ExternalInput")
    wt = nc.dram_tensor("wt", [F_IN, KF], f32, kind="ExternalInput")
    avec = nc.dram_tensor("avec", [2, KF], f32, kind="ExternalInput")
    idxin = nc.dram_tensor("idxin", [P, CTOT], i16, kind="ExternalInput")
    padfill = nc.dram_tensor("padfill", [sp - S if sp > S else 1, 4], f32,
                             kind="ExternalInput")
    out = nc.dram_tensor("out", [sp, KF], f32, kind="ExternalOutput")

    m_out = z_out = s_out = hfe_out = g0_out = es_out = hse_out = None
    JTMAX = int(Jt.max())
    if dump:
        m_out = nc.dram_tensor("m_out", [sp, K], f32, kind="ExternalOutput")
        z_out = nc.dram_tensor("z_out", [sp, K], f32, kind="ExternalOutput")
        s_out = nc.dram_tensor("s_out", [sp, K * JTMAX], f32,
                               kind="ExternalOutput")
        hfe_out = nc.dram_tensor("hfe_out", [NTAB, 4], f32,
                                 kind="ExternalOutput")
        tc0 = int(gsizes[0]) * int(Jt[0])
        g0_out = nc.dram_tensor("g0_out", [P, tc0 * 4], f32,
                                kind="ExternalOutput")
        es_out = nc.dram_tensor("es_out", [sp, K], f32,
                                kind="ExternalOutput")
        hse_out = nc.dram_tensor("hse_out", [sp, 4], f32,
                                 kind="ExternalOutput")
    he_shard = nc.dram_tensor("he_shard", [sp, ELF], f32, kind="Internal")
    he_full = nc.dram_tensor("he_full", [NTAB, ELF], f32, kind="Internal",
                             addr_space="Shared" if distributed else "Local")

    with tile.TileContext(nc) as tc, ExitStack() as ctx:
        consts = ctx.enter_context(tc.tile_pool(name="consts", bufs=1))
        sa = ctx.enter_context(tc.tile_pool(name="sa", bufs=4))
        sa_ps = ctx.enter_context(tc.tile_pool(name="sa_ps", bufs=3, space="PSUM"))
        sc = ctx.enter_context(tc.tile_pool(name="sc", bufs=2))
        scg = ctx.enter_context(tc.tile_pool(name="scg", bufs=3))
        sci = ctx.enter_context(tc.tile_pool(name="sci", bufs=3))

        ident = consts.tile([P, P], f32)
        make_identity(nc, ident[:])
        # rhs_cat = [wt | B]; B[i, s*K+k] = sum_f wt[i, k*32+f] * a_s[k, f]
        rhs_cat = consts.tile([P, RHS], f32)
        nc.sync.dma_start(rhs_cat[:, 0:KF], wt.ap())
        av_sb = consts.tile([P, 2 * KF], f32)
        nc.sync.dma_start(av_sb[:], bass.AP(avec, 0, [[0, P], [1, 2 * KF]]))
        tmp_b = consts.tile([P, 2 * KF], f32)
        for s in range(2):
            nc.vector.tensor_mul(
                tmp_b[:, s * KF:(s + 1) * KF], rhs_cat[:, 0:KF],
                av_sb[:, s * KF:(s + 1) * KF])
        nc.vector.reduce_sum(
            rhs_cat[:, KF:RHS],
            tmp_b[:].rearrange("p (s k f) -> p s k f", s=2, f=F_OUT),
            axis=mybir.AxisListType.X)
        es_sb = consts.tile([P, nt * K], f32)

        # ---- Stage A (two tiles per iteration; batched copies/IO) ----
        assert nt % 2 == 0
        for t2 in range(0, nt, 2):
            x_t2 = sa.tile([P, 2 * F_IN], f32, tag="x")
            xv = x_t2[:].rearrange("p (t f) -> p t f", f=F_IN)
            nc.sync.dma_start(
                xv, xs.ap()[t2 * P:(t2 + 2) * P, :]
                    .rearrange("(t p) f -> p t f", p=P))
            xt_ps2 = sa_ps.tile([P, 2 * P], f32, tag="xt")
            for ti in range(2):
                nc.tensor.transpose(out=xt_ps2[:, ti * P:(ti + 1) * P],
                                    in_=xv[:, ti, :], identity=ident[:])
            xt_sb2 = sa.tile([P, 2 * P], f32, tag="xt_sb")
            nc.scalar.copy(xt_sb2[:], xt_ps2[:])
            he_ps2 = sa_ps.tile([P, 2 * RHS], f32, tag="he")
            for ti in range(2):
                nc.tensor.matmul(he_ps2[:, ti * RHS:(ti + 1) * RHS],
                                 lhsT=xt_sb2[:, ti * P:(ti + 1) * P],
                                 rhs=rhs_cat[:], start=True, stop=True)
            he_t2 = sa.tile([P, 2 * ELF], f32, tag="het")
            hev = he_t2[:].rearrange("p (t e) -> p t e", e=ELF)
            hpv = he_ps2[:].rearrange("p (t r) -> p t r", r=RHS)
            nc.gpsimd.memset(he_t2[:], 0.0)
            nc.scalar.copy(hev[:, :, 0:HF].bitcast(bf16), hpv[:, :, 0:KF])
            nc.vector.tensor_copy(
                hev[:, :, HF:HF + K], hpv[:, :, KF + K:RHS])
            nc.vector.tensor_copy(
                es_sb[:, t2 * K:(t2 + 2) * K]
                    .rearrange("p (t k) -> p t k", k=K),
                hpv[:, :, KF:KF + K])
            nc.sync.dma_start(
                he_shard.ap()[t2 * P:(t2 + 2) * P, :]
                    .rearrange("(t p) e -> p t e", p=P), hev)
        # pad rows are window dummies: e_dst <- -1e30 (after slab writes)
        npad = sp - S
        if npad > 0:
            pf = consts.tile([npad, 4], f32)
            nc.sync.dma_start(pf[:], padfill.ap())
            nc.sync.dma_start(he_shard.ap()[S:sp, HF:HF + K], pf[:])

        # ---- Stage B ----
        if distributed:
            nc.gpsimd.collective_compute(
                "AllGather", mybir.AluOpType.bypass,
                replica_groups=[list(range(n_cores))],
                ins=[he_shard.ap()], outs=[he_full.ap()])
        else:
            for t in range(nt):
                cp = sa.tile([P, ELF], f32, tag="cp")
                nc.sync.dma_start(cp[:], he_shard.ap()[t * P:(t + 1) * P, :])
                nc.sync.dma_start(he_full.ap()[t * P:(t + 1) * P, :], cp[:])

        if dump:
            nc.sync.dma_start(
                hse_out.ap(),
                bass.AP(he_shard, HF, [[ELF, sp], [1, 4]]))
            HB = NTAB // 4
            for q in range(4):
                nc.sync.dma_start(
                    hfe_out.ap()[q * HB:(q + 1) * HB, :],
                    bass.AP(he_full, q * HB * ELF + HF, [[ELF, HB], [1, 4]]))
            nc.sync.dma_start(
                es_out.ap().rearrange("(t p) k -> p t k", p=P),
                es_sb[:].rearrange("p (t k) -> p t k", k=K))
        # ---- Stage C ----
        coff = 0
        ncall = 0
        tbase = 0
        for g in range(NG):
            gs = gsizes[g]
            jt = int(Jt[g])
            totcol = gs * jt
            idx_g = sci.tile([P, totcol * 8], i16, tag="idx")
            nc.sync.dma_start(idx_g[:], idxin.ap()[:, coff:coff + totcol * 8])
            coff += totcol * 8
            gbuf = scg.tile([P, totcol * ELF], f32, tag="g")
            g3 = gbuf[:].rearrange("p (c e) -> p c e", e=ELF)
            # window-major gather blocks: block w at column gs*sum(J[g,:w])
            bw = 0
            ioff = 0
            for w in range(NW):
                jw = int(J[g, w])
                if jw == 0:
                    continue
                nidx = gs * jw * P
                nc.gpsimd.dma_gather(
                    out_ap=g3[:, bw:bw + gs * jw, :],
                    in_ap=he_full.ap()[BOUNDS[w]:BOUNDS[w + 1], :],
                    idxs_ap=idx_g[:, ioff:ioff + gs * jw * 8],
                    num_idxs=nidx, num_idxs_reg=nidx, elem_size=ELF,
                    single_packet=False, queue_num=ncall % 4)
                ncall += 1
                bw += gs * jw
                ioff += gs * jw * 8
            if dump and g == 0:
                nc.sync.dma_start(g0_out.ap(), g3[:, :, HF:HF + K])
            # group-wide score assembly: sag[p, t, k, d]
            s_allg = sc.tile([P, gs * K * jt], f32, tag="s0")
            sag = s_allg[:].rearrange("p (t k d) -> p t k d", k=K, d=jt)
            esg = es_sb[:, tbase * K:(tbase + gs) * K].rearrange(
                "p (t k) -> p t k", k=K).unsqueeze(-1)
            bw = 0
            c0 = 0
            for w in range(NW):
                jw = int(J[g, w])
                if jw == 0:
                    continue
                eb = g3[:, bw:bw + gs * jw, HF:HF + K].rearrange(
                    "p (t d) k -> p t k d", d=jw)
                nc.vector.tensor_add(
                    sag[:, :, :, c0:c0 + jw], eb,
                    esg.to_broadcast([P, gs, K, jw]))
                bw += gs * jw
                c0 += jw
            s1g = sc.tile([P, gs * K * jt], f32, tag="s1")
            nc.vector.scalar_tensor_tensor(
                s1g[:], s_allg[:], NEG_SLOPE, s_allg[:],
                op0=mybir.AluOpType.mult, op1=mybir.AluOpType.max)
            s1v = s1g[:].rearrange("p (tk d) -> p tk d", d=jt)
            mg = sc.tile([P, gs * K], f32, tag="m")
            nc.vector.reduce_max(mg[:], s1v, axis=mybir.AxisListType.X)
            s2g = sc.tile([P, gs * K * jt], f32, tag="s2")
            nc.vector.tensor_sub(
                s2g[:].rearrange("p (tk d) -> p tk d", d=jt), s1v,
                mg[:].unsqueeze(-1).to_broadcast([P, gs * K, jt]))
            zg = sc.tile([P, gs * K], f32, tag="z")
            prs = sc.tile([P, gs * K * jt], f32, tag="prs")
            nc.scalar.activation(prs[:], s2g[:], AF.Exp)
            nc.vector.reduce_sum(
                zg[:], prs[:].rearrange("p (tk d) -> p tk d", d=jt),
                axis=mybir.AxisListType.X)
            vall = sc.tile([P, gs * KF], bf16, tag="vall")
            for ti in range(gs):
                t = tbase + ti
                # expanded exp on ACT: prx[p, d, (k f)] = exp(s2[p, k, d])
                prx = sc.tile([P, jt * KF], bf16, tag="prx")
                nc.scalar.activation(
                    prx[:].rearrange("p (d k f) -> p d k f", k=K, f=F_OUT),
                    s2g[:, ti * K * jt:(ti + 1) * K * jt]
                        .rearrange("p (k d) -> p d k", d=jt)
                        .unsqueeze(-1).to_broadcast([P, jt, K, F_OUT]),
                    AF.Exp)
                if dump:
                    nc.sync.dma_start(
                        s_out.ap()[t * P:(t + 1) * P, 0:K * jt],
                        s_allg[:, ti * K * jt:(ti + 1) * K * jt])
                # weighted h: wg[p, d, kf] = g_h * prx
                wg = sc.tile([P, jt * KF], bf16, tag="wg")
                wgv = wg[:].rearrange("p (d e) -> p d e", e=KF)
                prxv = prx[:].rearrange("p (d e) -> p d e", e=KF)
                bw = 0
                c0 = 0
                for w in range(NW):
                    jw = int(J[g, w])
                    if jw == 0:
                        continue
                    nc.vector.tensor_mul(
                        wgv[:, c0:c0 + jw, :],
                        g3[:, bw + ti * jw: bw + (ti + 1) * jw,
                           0:HF].bitcast(bf16),
                        prxv[:, c0:c0 + jw, :])
                    bw += gs * jw
                    c0 += jw
                # pairwise add tree over slots (bf16 2x mode); final level
                # lands in this tile's vall slice
                buf, width, lvl = wgv, jt, 0
                while width > 1:
                    h2, r = divmod(width, 2)
                    if h2 + r == 1:
                        nxt = vall[:, ti * KF:(ti + 1) * KF].rearrange(
                            "p (d e) -> p d e", e=KF)
                    else:
                        nxt_t = sc.tile([P, (h2 + r) * KF], bf16,
                                        tag=f"tr{lvl}")
                        nxt = nxt_t[:].rearrange("p (d e) -> p d e", e=KF)
                    nc.vector.tensor_add(
                        nxt[:, 0:h2, :],
                        buf[:, 0:2 * h2:2, :], buf[:, 1:2 * h2:2, :])
                    if r:
                        nc.vector.tensor_copy(
                            nxt[:, h2, :], buf[:, 2 * h2, :])
                    buf, width = nxt, h2 + r
                    lvl += 1
            # group-wide normalize + ELU + store
            rzg = sc.tile([P, gs * K], f32, tag="rz")
            nc.vector.reciprocal(rzg[:], zg[:])
            if dump:
                nc.sync.dma_start(
                    m_out.ap()[tbase * P:(tbase + gs) * P, :]
                        .rearrange("(t p) k -> p t k", p=P),
                    mg[:].rearrange("p (t k) -> p t k", k=K))
                nc.sync.dma_start(
                    z_out.ap()[tbase * P:(tbase + gs) * P, :]
                        .rearrange("(t p) k -> p t k", p=P),
                    zg[:].rearrange("p (t k) -> p t k", k=K))
            og = sc.tile([P, gs * KF], f32, tag="o")
            nc.vector.tensor_mul(
                og[:].rearrange("p (tk f) -> p tk f", f=F_OUT),
                vall[:].rearrange("p (tk f) -> p tk f", f=F_OUT),
                rzg[:].unsqueeze(-1).to_broadcast([P, gs * K, F_OUT]))
            t1 = sc.tile([P, gs * KF], f32, tag="t1")
            nc.vector.tensor_scalar_min(t1[:], og[:], 0.0)
            e1 = sc.tile([P, gs * KF], f32, tag="e1")
            nc.scalar.activation(e1[:], t1[:], AF.Exp)
            r_ = sc.tile([P, gs * KF], f32, tag="r")
            nc.vector.tensor_scalar_max(r_[:], og[:], 0.0)
            ot = sc.tile([P, gs * KF], f32, tag="ot")
            nc.vector.scalar_tensor_tensor(
                ot[:], e1[:], -1.0, r_[:],
                op0=mybir.AluOpType.add, op1=mybir.AluOpType.add)
            nc.sync.dma_start(
                out.ap()[tbase * P:(tbase + gs) * P, :]
                    .rearrange("(t p) f -> p t f", p=P),
                ot[:].rearrange("p (t f) -> p t f", f=KF))
            tbase += gs

    nc.compile()
    return nc


def _hilbert_d(coords, bits=5):
    """Hilbert distance of [n, d] integer coords (Skilling transpose)."""
    n, nd = coords.shape
    X = coords.T.astype(np.uint32).copy()
    M = np.uint32(1 << (bits - 1))
    Q = M
    while Q > 1:
        Pq = Q - 1
        for i in range(nd):
            t = (X[i] & Q) > 0
            X[0] = np.where(t, X[0] ^ Pq, X[0])
            m = np.where(t, np.uint32(0), Pq)
            tt = (X[0] ^ X[i]) & m
            X[0] ^= tt
            X[i] ^= tt
        Q >>= 1
    for i in range(1, nd):
        X[i] ^= X[i - 1]
    t2 = np.zeros(n, np.uint32)
    Q = M
    while Q > 1:
        t2 = np.where((X[nd - 1] & Q) > 0, t2 ^ (Q - 1), t2)
        Q >>= 1
    for i in range(nd):
        X[i] ^= t2
    out = np.zeros(n, np.int64)
    for b in range(bits):
        for i in range(nd):
            out |= np.int64(((X[i] >> (bits - 1 - b)) & 1).astype(np.int64)
                            ) << ((bits - 1 - b) * nd + (nd - 1 - i))
    return out


def host_plan(nbr):
    """Hilbert node ordering per core, per-group window slot table J,
    per-core idx buffers (window-major grouped gather layout)."""
    nbr = np.asarray(nbr).astype(np.int64)
    src_core = nbr // S
    win = src_core // 2                                     # [N, DEG] in 0..3
    orders = []
    cnts = []
    for c in range(N_CORES):
        w = win[c * S:(c + 1) * S]
        cnt = np.stack([(w == q).sum(1) for q in range(NW)], 1)  # [S, NW]
        order = np.argsort(_hilbert_d(cnt), kind="stable")
        orders.append(order)
        cnts.append(cnt)
    # table row of neighbor j = c_j*SP + inv_order_{c_j}(j % S)
    inv = np.empty(N, np.int64)
    for c in range(N_CORES):
        inv[c * S + orders[c]] = np.arange(S)
    rows = (src_core * SP + inv[nbr]).astype(np.int64)
    percore = []
    M = np.zeros((NT, NW), np.int64)    # per-tile cross-core window maxima
    for c in range(N_CORES):
        r = np.sort(rows[c * S:(c + 1) * S], axis=1)[orders[c]]  # [S, 16]
        cnt = cnts[c][orders[c]]
        rs = np.concatenate([r, np.zeros((SP - S, DEG), np.int64)])
        cs = np.concatenate([cnt, np.zeros((SP - S, NW), np.int64)])
        start = np.concatenate(
            [np.zeros((SP, 1), np.int64), np.cumsum(cs, 1)[:, :-1]], 1)
        percore.append((rs, cs, start))
        M = np.maximum(M, cs.reshape(NT, P, NW).max(1))
    # greedy variable grouping under the gather-buffer column cap
    groups = []
    t0 = 0
    while t0 < NT:
        gs = 1
        Jg = M[t0].copy()
        while t0 + gs < NT:
            Jn = np.maximum(Jg, M[t0 + gs])
            if (gs + 1) * Jn.sum() > GCAP:
                break
            Jg = Jn
            gs += 1
        groups.append((t0, gs, Jg))
        t0 += gs
    gsizes = [gs for _, gs, _ in groups]
    J = np.stack([Jg for _, _, Jg in groups])               # [ng, NW]
    NG = len(groups)
    idxbufs = []
    for c in range(N_CORES):
        rs, cs, start = percore[c]
        segs = []
        for g in range(NG):
            t0g, gs, _ = groups[g]
            lo = t0g * P
            for w in range(NW):
                jw = int(J[g, w])
                if jw == 0:
                    continue
                rt = rs[lo:lo + gs * P]                    # [gs*128, 16]
                ct = cs[lo:lo + gs * P, w:w + 1]
                st = start[lo:lo + gs * P, w:w + 1]
                jj = np.arange(jw)[None, :]
                take = st + jj
                valid = jj < ct
                vals = np.where(
                    valid,
                    np.take_along_axis(
                        rt, np.minimum(take, DEG - 1).astype(np.int64), 1),
                    DUMMY[w]).astype(np.int64) - BOUNDS[w]
                # [gs*128, jw] -> stream: for t: for j: for p
                v3 = vals.reshape(gs, P, jw).transpose(0, 2, 1)
                lin = v3.reshape(-1)                        # [gs*jw*128]
                seg = lin.reshape(-1, 16).T.astype(np.int16)  # [16, gs*jw*8]
                segs.append(seg)
        buf16 = np.concatenate(segs, axis=1)
        idxbufs.append(np.ascontiguousarray(np.tile(buf16, (8, 1))))
    return J, gsizes, orders, idxbufs


def prep_inputs(X, W, a, nbr):
    X = np.asarray(X, dtype=np.float32)
    W = np.asarray(W, dtype=np.float32)
    a = np.asarray(a, dtype=np.float32)
    J, gsizes, orders, idxbufs = host_plan(nbr)
    wt = np.ascontiguousarray(W.transpose(2, 0, 1).reshape(F_IN, KF))
    avec = np.ascontiguousarray(
        np.stack([a[:, 0, :F_OUT].reshape(KF), a[:, 0, F_OUT:].reshape(KF)]))
    pf = np.ascontiguousarray(
        np.full((max(SP - S, 1), 4), NEG_BIG, dtype=np.float32))
    in_maps = []
    for c in range(N_CORES):
        xs = np.zeros((SP, F_IN), dtype=np.float32)
        xs[:S] = X[c * S:(c + 1) * S][orders[c]]
        in_maps.append({"xs": xs, "wt": wt, "avec": avec, "idxin": idxbufs[c],
                        "padfill": pf})
    return J, gsizes, orders, in_maps


_NC_CACHE = {}


def kernel(X, W, a, nbr):
    from concourse.bass_utils import run_bass_kernel_spmd

    J, gsizes, orders, in_maps = prep_inputs(X, W, a, nbr)
    key = hashlib.sha1(
        J.tobytes() + np.asarray(gsizes, np.int64).tobytes()).hexdigest()
    if key not in _NC_CACHE:
        _NC_CACHE[key] = build_nc(J, gsizes)
    nc = _NC_CACHE[key]
    res = run_bass_kernel_spmd(nc, in_maps, core_ids=list(range(N_CORES)))
    out = np.empty((N, KF), dtype=np.float32)
    for c in range(N_CORES):
        out[c * S + orders[c]] = res.results[c]["out"][:S]
    return out
